# revision 1
# baseline (speedup 1.0000x reference)
"""Trainium2 Bass kernel for nn_DenseTensor (dense_mlp, bilinear form).

Computes out = x @ W + einsum('bd,due,be->bu', x, V, x) + b with
B=1024, D=U=E=512 on 8 NeuronCores.

Sharding: tensor-parallel over the units axis U — core c owns units
[c*64, (c+1)*64). Each core receives the full x (replicated, as a bf16
x^T for the matmul stationary operand plus an f32 x for the reduce
stage) and its V/W/b shard. No collectives; the host concatenates the
8 disjoint output column-slices.

Per-core dataflow, per unit u:
  PE : A_u = x @ V[:,u,:]  as 8 batch-chunks x 4 K-chunk accumulating
       bf16 matmuls ([128k,128m] @ [128k,512n] -> PSUM f32).
  DVE: one fused tensor_tensor_reduce per batch-chunk:
       quad[b,u] = sum_e A_u[b,e] * x[b,e]  (product + row-reduce in a
       single pass, accumulated straight into the output column).
The linear term x @ W_shard + b is computed once up front (PE + DVE)
and added into the output tile with a single tensor_add at the end.
"""

import sys
import types

import numpy as np
import ml_dtypes

B, D, U = 1024, 512, 512
N_CORES = 8
UPC = U // N_CORES       # units per core = 64
P = 128                  # partitions
BC = B // P              # batch chunks = 8
KC = D // P              # contraction chunks = 4

BF16 = ml_dtypes.bfloat16


def _ensure_axon_hooks():
    """Provide the antenv.axon_hooks registry if the image lacks it.

    concourse.bass_utils imports it unconditionally when tracing is
    requested (e.g. BASS_TRACE=1); without this shim that import path
    raises ModuleNotFoundError.
    """
    try:
        import antenv.axon_hooks  # noqa: F401
        return
    except ImportError:
        pass
    mod = types.ModuleType("antenv.axon_hooks")
    mod._hook = None

    def set_axon_ntff_profile_hook(h):
        mod._hook = h

    def get_axon_ntff_profile_hook():
        return mod._hook

    mod.set_axon_ntff_profile_hook = set_axon_ntff_profile_hook
    mod.get_axon_ntff_profile_hook = get_axon_ntff_profile_hook
    sys.modules["antenv.axon_hooks"] = mod
    try:
        import antenv
        antenv.axon_hooks = mod
    except ImportError:
        pass
    try:
        from trn_agent_boot.trn_boot import _ntff_profile_via_ctypes
        hook = _ntff_profile_via_ctypes("/opt/axon/libaxon_pjrt.so")
        if hook is not None:
            set_axon_ntff_profile_hook(hook)
    except Exception:
        pass


def _split_multi_waits(nc, mybir, max_waits=1):
    """Legalize for walrus builds that allow only one sync wait per
    instruction: move extra waits onto same-engine NoOps placed just
    before the offending instruction (queues are in-order, so this is
    semantics-preserving)."""
    for f in nc.m.functions:
        for blk in f.blocks:
            new_insts, changed = [], False
            for inst in blk.instructions:
                si = inst.sync_info
                if si is not None and len(si.on_wait) > max_waits:
                    waits = list(si.on_wait)
                    extra, keep = waits[:-max_waits], waits[-max_waits:]
                    for j, w in enumerate(extra):
                        new_insts.append(mybir.InstNoOp(
                            name=f"{inst.name}-sw{j}",
                            engine=inst.engine,
                            bass_nofuse=True,
                            sync_info=mybir.SyncInfo(on_wait=[w], on_update=[]),
                        ))
                    inst.sync_info = mybir.SyncInfo(
                        on_wait=keep, on_update=list(si.on_update))
                    changed = True
                new_insts.append(inst)
            if changed:
                blk.instructions = new_insts


def _build_program():
    import concourse.bass as bass
    import concourse.mybir as mybir
    import concourse.tile as tile

    f32 = mybir.dt.float32
    bf16 = mybir.dt.bfloat16

    nc = bass.Bass(trn_type="TRN2")
    xT = nc.dram_tensor("xT", [D, B], bf16, kind="ExternalInput")
    x32 = nc.dram_tensor("x32", [B, D], f32, kind="ExternalInput")
    Vs = nc.dram_tensor("Vs", [UPC, P, KC, D], bf16, kind="ExternalInput")
    Ws = nc.dram_tensor("Ws", [D, UPC], bf16, kind="ExternalInput")
    bs = nc.dram_tensor("bs", [P, UPC], f32, kind="ExternalInput")
    outs = nc.dram_tensor("outs", [B, UPC], f32, kind="ExternalOutput")

    mult = mybir.AluOpType.mult
    add = mybir.AluOpType.add

    with tile.TileContext(nc) as tc:
        with tc.tile_pool(name="const", bufs=1) as cpool:
            xT_sb = cpool.tile([P, KC, B], bf16)
            x32_sb = cpool.tile([P, BC, D], f32)
            ws_sb = cpool.tile([P, KC, UPC], bf16)
            bias_sb = cpool.tile([P, UPC], f32)
            lin_sb = cpool.tile([P, BC, UPC], f32)
            out_sb = cpool.tile([P, BC, UPC], f32)

            # DMA plan: the first two unit-pairs' V tiles go FIRST on the
            # sync queue (they gate the first quad matmuls), then the xT
            # chunks; x32/Ws/bias ride the gpsimd queue in parallel.
            xT_r = xT.rearrange("(k p) b -> p k b", p=P)

            # Units in pairs (G=2): each pair's matmuls land in one
            # [P, 2, D] PSUM tile (2 banks), 4 tiles in flight so the PE
            # never waits on the stage-2 consumer latency. Stage 2 per
            # pair: one broadcast product over both banks (DVE, fixed
            # overhead amortized), then row-reduces — mostly on the
            # Scalar engine (accumulate-activation), every 5th pair as a
            # batched reduce on DVE — keeping both engines well under the
            # PE's cadence.
            G = 2
            Copy = mybir.ActivationFunctionType.Copy
            with tc.tile_pool(name="vp", bufs=3 * G) as vpool, \
                 tc.tile_pool(name="qp", bufs=4, space="PSUM") as qpool, \
                 tc.tile_pool(name="dp", bufs=1) as dpool:
                act_dummy = dpool.tile([P, D], bf16)

                def v_load(u):
                    # Host pre-permuted V to [p, k, e]: one contiguous 4KB
                    # line per partition per unit -> fast uniform DMAs.
                    vt = vpool.tile([P, KC, D], bf16, tag="vt")
                    nc.sync.dma_start(out=vt, in_=Vs[u])
                    return vt

                PRE = 2
                pre_vts = {ug: [v_load(ug * G + j) for j in range(G)]
                           for ug in range(PRE)}
                for k in range(KC):
                    nc.sync.dma_start(out=xT_sb[:, k, :], in_=xT_r[:, k, :])
                nc.gpsimd.dma_start(
                    out=ws_sb, in_=Ws.rearrange("(k p) u -> p k u", p=P))
                nc.gpsimd.dma_start(
                    out=x32_sb, in_=x32.rearrange("(c p) d -> p c d", p=P))
                nc.gpsimd.dma_start(out=bias_sb, in_=bs[:, :])

                gidx = 0
                for ug in range(UPC // G):
                    vts = pre_vts.pop(ug) if ug in pre_vts else \
                        [v_load(ug * G + j) for j in range(G)]
                    for bc in range(BC):
                        qg = qpool.tile([P, G, D], f32)
                        for k in range(KC):
                            for j in range(G):
                                nc.tensor.matmul(
                                    qg[:, j, :],
                                    xT_sb[:, k, bc * P:(bc + 1) * P],
                                    vts[j][:, k, :],
                                    start=(k == 0),
                                    stop=(k == KC - 1),
                                )
                        xb = x32_sb[:, bc, :][:, None, :].broadcast_to((P, G, D))
                        nc.vector.tensor_mul(qg, qg, xb)
                        u0 = ug * G
                        if gidx % 5 == 0:
                            nc.vector.tensor_reduce(
                                out_sb[:, bc, u0:u0 + G], qg,
                                mybir.AxisListType.X, add)
                        else:
                            for j in range(G):
                                nc.scalar.activation(
                                    act_dummy, qg[:, j, :], Copy,
                                    accum_out=out_sb[:, bc, u0 + j:u0 + j + 1])
                        gidx += 1

                # Linear term last: its small matmuls run on the PE while
                # the final quad groups drain on DVE/ACT (PE is otherwise
                # idle there). Reuses the quad PSUM slots (same tag) to
                # stay within the 8-bank budget.
                for bc in range(BC):
                    lp = qpool.tile([P, G, D], f32, tag="qg")
                    for k in range(KC):
                        nc.tensor.matmul(
                            lp[:, 0, 0:UPC],
                            xT_sb[:, k, bc * P:(bc + 1) * P],
                            ws_sb[:, k, :],
                            start=(k == 0),
                            stop=(k == KC - 1),
                        )
                    nc.vector.tensor_add(
                        lin_sb[:, bc, :], lp[:, 0, 0:UPC], bias_sb)

            nc.vector.tensor_add(out_sb, out_sb, lin_sb)
            nc.sync.dma_start(
                out=outs.rearrange("(c p) u -> p c u", p=P), in_=out_sb)

    _split_multi_waits(nc, mybir, max_waits=1)
    return nc


_LAST_RUN = {}


def kernel(x, W, V, b):
    _ensure_axon_hooks()
    import concourse.bass_utils as bass_utils
    bass_utils.upload_artifacts = lambda d: f"local:{d}"

    x = np.asarray(x, dtype=np.float32)
    W = np.asarray(W, dtype=np.float32)
    V = np.asarray(V, dtype=np.float32)
    b = np.asarray(b, dtype=np.float32)

    xT_bf = np.ascontiguousarray(x.T).astype(BF16)
    Vt_bf = V.transpose(1, 0, 2).astype(BF16)   # (U, D, E) contiguous bf16
    # Permute each unit's matrix to [p, k, e] (partition-major for the
    # K-chunked matmul layout) so the per-unit DMA is contiguous.
    Vt_bf = Vt_bf.reshape(U, KC, P, D).transpose(0, 2, 1, 3)
    W_bf = W.astype(BF16)

    in_maps = []
    for c in range(N_CORES):
        us, ue = c * UPC, (c + 1) * UPC
        in_maps.append({
            "xT": xT_bf,
            "x32": x,
            "Vs": np.ascontiguousarray(Vt_bf[us:ue]),
            "Ws": np.ascontiguousarray(W_bf[:, us:ue]),
            "bs": np.ascontiguousarray(
                np.broadcast_to(b[us:ue], (P, UPC))).astype(np.float32),
        })

    nc = _build_program()
    res = None
    last_exc = None
    for attempt in range(3):
        try:
            res = bass_utils.run_bass_kernel_spmd(
                nc, in_maps, core_ids=list(range(N_CORES)))
            break
        except Exception as e:  # transient NRT device errors have been seen
            last_exc = e
    if res is None:
        raise last_exc
    _LAST_RUN["result"] = res

    out = np.concatenate(
        [res.results[c]["outs"] for c in range(N_CORES)], axis=1)
    return out.astype(np.float32)



# revision 4
# speedup vs baseline: 1.7092x; 1.7092x over previous
"""Trainium2 Bass kernel for nn_DenseTensor (dense_mlp, bilinear form).

Computes out = x @ W + einsum('bd,due,be->bu', x, V, x) + b with
B=1024, D=U=E=512 on 8 NeuronCores.

Algorithm: the quadratic form depends only on the symmetric part of V
in (d,e), so the D*E contraction is folded to D(D+1)/2 unordered pairs
enumerated by circulant offset o: pair (d, e=(d+o)%512) for o in
[0,256].  Host pre-folds coefficients Sh[(o,d),u] = V[d,u,e]+V[e,u,d]
(o=0 diag: V[d,u,d]; o=256: d<256 only).  This HALVES the PE FLOPs vs
the naive einsum.

Sharding: by contraction (the o axis) - core c owns ~32 consecutive o
values; every core computes a partial full [U,B] output and the host
sums the 8 partials (the unshard step for contraction sharding).  The
linear term x@W + b is computed by all cores at 1/8 weight so the sum
is exact, keeping the SPMD program identical across cores.

Per-core, per o-unit:
  DMA : xw = 512-row window of the wrap-extended x^T at offset o
        (one contiguous slab per k-chunk - no on-device shuffles),
        plus the unit's coefficient block Sh[o] ([128,4,512] bf16).
  DVE : G = xT .* xw   (pair products, one tensor_mul, bf16 2x mode)
  PE  : for k in 4, ub in 4, h in 2: matmul(acc[ub][h] += Sh_chunk^T @
        G_chunk), N=512, accumulating in 8 PSUM banks over the whole
        kernel (start on first unit, stop on last).
Tail: acc + bias/8 -> SBUF -> DMA out.
"""

import sys
import types

import numpy as np
import ml_dtypes

B, D, U = 1024, 512, 512
N_CORES = 8
P = 128                  # partitions
KC = D // P              # k-chunks per unit = 4
NO = 257                 # circulant offsets 0..256
NUNIT = 34               # 33 o-units + 1 W-unit (uniform across cores)
# per-core o ranges: sizes sum to 257, each <= 33
O_SIZES = [32, 32, 32, 32, 33, 32, 32, 32]
O_START = [0, 32, 64, 96, 128, 161, 193, 225]

BF16 = ml_dtypes.bfloat16


def _ensure_axon_hooks():
    """Provide the antenv.axon_hooks registry if the image lacks it."""
    try:
        import antenv.axon_hooks  # noqa: F401
        return
    except ImportError:
        pass
    mod = types.ModuleType("antenv.axon_hooks")
    mod._hook = None

    def set_axon_ntff_profile_hook(h):
        mod._hook = h

    def get_axon_ntff_profile_hook():
        return mod._hook

    mod.set_axon_ntff_profile_hook = set_axon_ntff_profile_hook
    mod.get_axon_ntff_profile_hook = get_axon_ntff_profile_hook
    sys.modules["antenv.axon_hooks"] = mod
    try:
        import antenv
        antenv.axon_hooks = mod
    except ImportError:
        pass
    try:
        from trn_agent_boot.trn_boot import _ntff_profile_via_ctypes
        hook = _ntff_profile_via_ctypes("/opt/axon/libaxon_pjrt.so")
        if hook is not None:
            set_axon_ntff_profile_hook(hook)
    except Exception:
        pass


def _split_multi_waits(nc, mybir, max_waits=1):
    """Legalize for walrus builds that allow only one sync wait per
    instruction: move extra waits onto same-engine NoOps placed just
    before the offending instruction (queues are in-order, so this is
    semantics-preserving)."""
    for f in nc.m.functions:
        for blk in f.blocks:
            new_insts, changed = [], False
            for inst in blk.instructions:
                si = inst.sync_info
                if si is not None and len(si.on_wait) > max_waits:
                    waits = list(si.on_wait)
                    extra, keep = waits[:-max_waits], waits[-max_waits:]
                    for j, w in enumerate(extra):
                        new_insts.append(mybir.InstNoOp(
                            name=f"{inst.name}-sw{j}",
                            engine=inst.engine,
                            bass_nofuse=True,
                            sync_info=mybir.SyncInfo(on_wait=[w], on_update=[]),
                        ))
                    inst.sync_info = mybir.SyncInfo(
                        on_wait=keep, on_update=list(si.on_update))
                    changed = True
                new_insts.append(inst)
            if changed:
                blk.instructions = new_insts


def _build_program():
    import concourse.bass as bass
    import concourse.mybir as mybir
    import concourse.tile as tile

    f32 = mybir.dt.float32
    bf16 = mybir.dt.bfloat16

    nc = bass.Bass(trn_type="TRN2")
    xTc = nc.dram_tensor("xTc", [P, KC, B], bf16, kind="ExternalInput")
    xE = nc.dram_tensor("xE", [544, B], bf16, kind="ExternalInput")
    Sh = nc.dram_tensor("Sh", [NUNIT, P, KC, U], bf16, kind="ExternalInput")
    bias_bc = nc.dram_tensor("bias_bc", [P, KC, 512], f32,
                             kind="ExternalInput")
    outs = nc.dram_tensor("outs", [U, B], f32, kind="ExternalOutput")

    with tile.TileContext(nc) as tc:
        with tc.tile_pool(name="const", bufs=1) as cpool:
            xT_sb = cpool.tile([P, KC, B], bf16)
            bias_sb = cpool.tile([P, KC, 512], f32)
            out_sb = cpool.tile([P, KC, B], f32)
            nc.sync.dma_start(out=xT_sb, in_=xTc[:, :, :])
            nc.gpsimd.dma_start(out=bias_sb, in_=bias_bc[:, :, :])

            with tc.tile_pool(name="wp", bufs=3) as wpool, \
                 tc.tile_pool(name="gp", bufs=3) as gpool, \
                 tc.tile_pool(name="sp", bufs=4) as spool, \
                 tc.tile_pool(name="ap", bufs=1, space="PSUM") as apool:
                accs = [[None, None] for _ in range(4)]
                for ub in range(4):
                    for h in range(2):
                        acc_t = apool.tile([P, 512], f32, tag=f"acc{ub}_{h}")
                        accs[ub][h] = acc_t
                for j in range(NUNIT):
                    if j < NUNIT - 1:
                        xw = wpool.tile([P, KC, B], bf16, tag="xw")
                        for k in range(KC):
                            nc.sync.dma_start(
                                out=xw[:, k, :],
                                in_=xE[j + P * k: j + P * (k + 1), :])
                        g = gpool.tile([P, KC, B], bf16, tag="g")
                        nc.vector.tensor_mul(g, xT_sb, xw)
                    else:
                        g = xT_sb   # W-unit: moving operand is x^T itself
                    sh = spool.tile([P, KC, U], bf16, tag="sh")
                    nc.scalar.dma_start(out=sh, in_=Sh[j])
                    for k in range(KC):
                        for ub in range(4):
                            for h in range(2):
                                nc.tensor.matmul(
                                    accs[ub][h],
                                    sh[:, k, ub * P:(ub + 1) * P],
                                    g[:, k, h * 512:(h + 1) * 512],
                                    start=(j == 0 and k == 0),
                                    stop=(j == NUNIT - 1 and k == KC - 1),
                                )
                for ub in range(4):
                    for h in range(2):
                        nc.vector.tensor_add(
                            out_sb[:, ub, h * 512:(h + 1) * 512],
                            accs[ub][h], bias_sb[:, ub, :])

            nc.sync.dma_start(
                out=outs.rearrange("(ub p) b -> p ub b", p=P), in_=out_sb)

    _split_multi_waits(nc, mybir, max_waits=1)
    return nc


def _host_inputs(x, W, V, b):
    """Build the per-core input arrays (all host-side prep)."""
    xT_bf = np.ascontiguousarray(x.T).astype(BF16)          # [D, B]
    xT_ext = np.concatenate([xT_bf, xT_bf[:288]], axis=0)   # [800, B]
    xTc_np = np.ascontiguousarray(
        xT_bf.reshape(KC, P, B).transpose(1, 0, 2))         # [P, KC, B]

    # folded symmetric coefficients
    Vt = V.transpose(0, 2, 1)                               # [d, e, u]
    Ssum = Vt + Vt.transpose(1, 0, 2)                       # V[d,u,e]+V[e,u,d]
    dd = np.arange(D)
    Vdiag = V[dd, :, dd]                                    # [d, u]

    W8 = (W.astype(np.float32) / N_CORES)
    bias_np = np.ascontiguousarray(np.broadcast_to(
        (b.astype(np.float32) / N_CORES).reshape(KC, P).T[:, :, None],
        (P, KC, 512))).astype(np.float32)

    def unit_block(M):      # [d, u] -> [p, k, u]
        return M.reshape(KC, P, U).transpose(1, 0, 2)

    in_maps = []
    for c in range(N_CORES):
        Sh_np = np.zeros((NUNIT, P, KC, U), dtype=np.float32)
        for j in range(O_SIZES[c]):
            o = O_START[c] + j
            if o == 0:
                M = Vdiag
            else:
                M = Ssum[dd, (dd + o) % D, :]
                if o == 256:
                    M = M.copy()
                    M[256:] = 0.0
            Sh_np[j] = unit_block(M)
        Sh_np[NUNIT - 1] = unit_block(W8)
        in_maps.append({
            "xTc": xTc_np,
            "xE": np.ascontiguousarray(xT_ext[O_START[c]:O_START[c] + 544]),
            "Sh": Sh_np.astype(BF16),
            "bias_bc": bias_np,
        })
    return in_maps


_LAST_RUN = {}


def kernel(x, W, V, b):
    _ensure_axon_hooks()
    import concourse.bass_utils as bass_utils
    bass_utils.upload_artifacts = lambda d: f"local:{d}"

    x = np.asarray(x, dtype=np.float32)
    W = np.asarray(W, dtype=np.float32)
    V = np.asarray(V, dtype=np.float32)
    b = np.asarray(b, dtype=np.float32)

    in_maps = _host_inputs(x, W, V, b)

    nc = _build_program()
    res = None
    last_exc = None
    for attempt in range(3):
        try:
            res = bass_utils.run_bass_kernel_spmd(
                nc, in_maps, core_ids=list(range(N_CORES)))
            break
        except Exception as e:  # transient NRT device errors have been seen
            last_exc = e
    if res is None:
        raise last_exc
    _LAST_RUN["result"] = res

    acc = np.zeros((U, B), dtype=np.float64)
    for c in range(N_CORES):
        acc += res.results[c]["outs"]
    return np.ascontiguousarray(acc.T).astype(np.float32)


# revision 5
# speedup vs baseline: 1.8938x; 1.1080x over previous
"""Trainium2 Bass kernel for nn_DenseTensor (dense_mlp, bilinear form).

Computes out = x @ W + einsum('bd,due,be->bu', x, V, x) + b with
B=1024, D=U=E=512 on 8 NeuronCores.

Algorithm: the quadratic form depends only on the symmetric part of V
in (d,e), so the D*E contraction folds to D(D+1)/2 unordered pairs
enumerated by circulant offset o: pair (d, e=(d+o)%512) for o in
[0,256].  Host pre-folds coefficients Sh[(o,d),u] = V[d,u,e]+V[e,u,d]
(o=0 diag: V[d,u,d]).  This HALVES the PE FLOPs vs the naive einsum.

Sharding: by contraction - core c owns offsets o in [32c, 32c+32);
every core computes a partial full [U,B] output and the host sums the
8 partials (the unshard step for contraction sharding).  Leftover work
rides one extra single-chunk "mini" unit per core, SPMD-uniform with
per-core data only:
  cores 0-3 : linear term chunk  (minA = x^T rows, minB = ones,
              coeff = W rows)
  core  4   : bias as rank-1     (minA = minB = ones, coeff row0 = b)
  core  5   : idle (zero coeffs)
  cores 6,7 : the o=256 half-offset pairs
Per-core, per o-unit:
  DMA : xw = 512-row window of the wrap-extended x^T at offset o
        (contiguous slabs - no on-device shuffles) + coeff block.
  DVE : G = xT .* xw  (pair products, one bf16 tensor_mul)
  PE  : 4 k-chunks x 4 u-blocks x 2 b-halves matmuls (N=512)
        accumulating into 8 PSUM banks across the whole kernel.
Tail: last unit loops accumulator-major so PSUM banks finish
staggered; evac copies alternate Vector/Scalar and stream out per
slice.  129 chunks/core = 1032 matmuls ~ 223 us of pure PE at the
warm roofline (216 ns per 128x128x512 bf16 matmul).
"""

import sys
import types

import numpy as np
import ml_dtypes

B, D, U = 1024, 512, 512
N_CORES = 8
P = 128                  # partitions
KC = D // P              # k-chunks per unit = 4
NJ = 32                  # o-units per core

BF16 = ml_dtypes.bfloat16


def _ensure_axon_hooks():
    """Provide the antenv.axon_hooks registry if the image lacks it."""
    try:
        import antenv.axon_hooks  # noqa: F401
        return
    except ImportError:
        pass
    mod = types.ModuleType("antenv.axon_hooks")
    mod._hook = None

    def set_axon_ntff_profile_hook(h):
        mod._hook = h

    def get_axon_ntff_profile_hook():
        return mod._hook

    mod.set_axon_ntff_profile_hook = set_axon_ntff_profile_hook
    mod.get_axon_ntff_profile_hook = get_axon_ntff_profile_hook
    sys.modules["antenv.axon_hooks"] = mod
    try:
        import antenv
        antenv.axon_hooks = mod
    except ImportError:
        pass
    try:
        from trn_agent_boot.trn_boot import _ntff_profile_via_ctypes
        hook = _ntff_profile_via_ctypes("/opt/axon/libaxon_pjrt.so")
        if hook is not None:
            set_axon_ntff_profile_hook(hook)
    except Exception:
        pass


def _split_multi_waits(nc, mybir, max_waits=1):
    """Legalize for walrus builds that allow only one sync wait per
    instruction: move extra waits onto same-engine NoOps placed just
    before the offending instruction (queues are in-order, so this is
    semantics-preserving)."""
    for f in nc.m.functions:
        for blk in f.blocks:
            new_insts, changed = [], False
            for inst in blk.instructions:
                si = inst.sync_info
                if si is not None and len(si.on_wait) > max_waits:
                    waits = list(si.on_wait)
                    extra, keep = waits[:-max_waits], waits[-max_waits:]
                    for j, w in enumerate(extra):
                        new_insts.append(mybir.InstNoOp(
                            name=f"{inst.name}-sw{j}",
                            engine=inst.engine,
                            bass_nofuse=True,
                            sync_info=mybir.SyncInfo(on_wait=[w], on_update=[]),
                        ))
                    inst.sync_info = mybir.SyncInfo(
                        on_wait=keep, on_update=list(si.on_update))
                    changed = True
                new_insts.append(inst)
            if changed:
                blk.instructions = new_insts


def _build_program():
    import concourse.bass as bass
    import concourse.mybir as mybir
    import concourse.tile as tile

    f32 = mybir.dt.float32
    bf16 = mybir.dt.bfloat16
    Copy = mybir.ActivationFunctionType.Copy

    nc = bass.Bass(trn_type="TRN2")
    xTc = nc.dram_tensor("xTc", [P, KC, B], bf16, kind="ExternalInput")
    xE = nc.dram_tensor("xE", [544, B], bf16, kind="ExternalInput")
    Sh = nc.dram_tensor("Sh", [NJ, P, KC, U], bf16, kind="ExternalInput")
    minA = nc.dram_tensor("minA", [P, B], bf16, kind="ExternalInput")
    minB = nc.dram_tensor("minB", [P, B], bf16, kind="ExternalInput")
    shMini = nc.dram_tensor("shMini", [P, U], bf16, kind="ExternalInput")
    outs = nc.dram_tensor("outs", [U, B], f32, kind="ExternalOutput")

    with tile.TileContext(nc) as tc:
        with tc.tile_pool(name="const", bufs=1) as cpool:
            xT_sb = cpool.tile([P, KC, B], bf16)
            out_sb = cpool.tile([P, KC, B], f32)
            ma_sb = cpool.tile([P, B], bf16)
            mb_sb = cpool.tile([P, B], bf16)
            ms_sb = cpool.tile([P, U], bf16)
            gm_sb = cpool.tile([P, B], bf16)

            # mini unit first: tiny loads so the PE starts almost
            # immediately while the big unit-0 windows stream in.
            nc.sync.dma_start(out=ma_sb, in_=minA[:, :])
            nc.sync.dma_start(out=mb_sb, in_=minB[:, :])
            nc.scalar.dma_start(out=ms_sb, in_=shMini[:, :])

            with tc.tile_pool(name="wp", bufs=3) as wpool, \
                 tc.tile_pool(name="gp", bufs=3) as gpool, \
                 tc.tile_pool(name="sp", bufs=4) as spool, \
                 tc.tile_pool(name="ap", bufs=1, space="PSUM") as apool:
                accs = [[None, None] for _ in range(4)]
                for ub in range(4):
                    for h in range(2):
                        acc_t = apool.tile([P, 512], f32, tag=f"acc{ub}_{h}")
                        accs[ub][h] = acc_t

                nc.vector.tensor_mul(gm_sb, ma_sb, mb_sb)
                for ub in range(4):
                    for h in range(2):
                        nc.tensor.matmul(
                            accs[ub][h],
                            ms_sb[:, ub * P:(ub + 1) * P],
                            gm_sb[:, h * 512:(h + 1) * 512],
                            start=True, stop=False)

                # unit 0: everything split per-k so the first window
                # chunk reaches the PE as soon as possible.
                xw0 = wpool.tile([P, KC, B], bf16, tag="xw")
                g0 = gpool.tile([P, KC, B], bf16, tag="g")
                sh0 = spool.tile([P, KC, U], bf16, tag="sh")
                for k in range(KC):
                    nc.sync.dma_start(out=xT_sb[:, k, :], in_=xTc[:, k, :])
                    nc.sync.dma_start(
                        out=xw0[:, k, :], in_=xE[P * k: P * (k + 1), :])
                    nc.scalar.dma_start(out=sh0[:, k, :], in_=Sh[0, :, k, :])
                    nc.vector.tensor_mul(
                        g0[:, k, :], xT_sb[:, k, :], xw0[:, k, :])
                for k in range(KC):
                    for ub in range(4):
                        for h in range(2):
                            nc.tensor.matmul(
                                accs[ub][h],
                                sh0[:, k, ub * P:(ub + 1) * P],
                                g0[:, k, h * 512:(h + 1) * 512],
                                start=False, stop=False)

                for j in range(1, NJ):
                    xw = wpool.tile([P, KC, B], bf16, tag="xw")
                    for k in range(KC):
                        nc.sync.dma_start(
                            out=xw[:, k, :],
                            in_=xE[j + P * k: j + P * (k + 1), :])
                    g = gpool.tile([P, KC, B], bf16, tag="g")
                    nc.vector.tensor_mul(g, xT_sb, xw)
                    sh = spool.tile([P, KC, U], bf16, tag="sh")
                    nc.scalar.dma_start(out=sh, in_=Sh[j])
                    if j < NJ - 1:
                        for k in range(KC):
                            for ub in range(4):
                                for h in range(2):
                                    nc.tensor.matmul(
                                        accs[ub][h],
                                        sh[:, k, ub * P:(ub + 1) * P],
                                        g[:, k, h * 512:(h + 1) * 512],
                                        start=False, stop=False)
                    else:
                        # last unit: accumulator-major so PSUM banks
                        # retire staggered and evac overlaps the tail.
                        for ub in range(4):
                            for h in range(2):
                                for k in range(KC):
                                    nc.tensor.matmul(
                                        accs[ub][h],
                                        sh[:, k, ub * P:(ub + 1) * P],
                                        g[:, k, h * 512:(h + 1) * 512],
                                        start=False, stop=(k == KC - 1))

                outs_r = outs.rearrange("(ub p) b -> p ub b", p=P)
                i = 0
                for ub in range(4):
                    for h in range(2):
                        dst = out_sb[:, ub, h * 512:(h + 1) * 512]
                        if i % 2 == 0:
                            nc.vector.tensor_copy(dst, accs[ub][h])
                        else:
                            nc.scalar.activation(dst, accs[ub][h], Copy)
                        nc.sync.dma_start(
                            out=outs_r[:, ub, h * 512:(h + 1) * 512],
                            in_=dst)
                        i += 1

    _split_multi_waits(nc, mybir, max_waits=1)
    return nc


def _host_inputs(x, W, V, b):
    """Build the per-core input arrays (all host-side prep)."""
    xT_bf = np.ascontiguousarray(x.T).astype(BF16)          # [D, B]
    xT_ext = np.concatenate([xT_bf, xT_bf[:256]], axis=0)   # [768, B]
    xTc_np = np.ascontiguousarray(
        xT_bf.reshape(KC, P, B).transpose(1, 0, 2))         # [P, KC, B]

    # folded symmetric coefficients
    Vt = V.transpose(0, 2, 1)                               # [d, e, u]
    Ssum = Vt + Vt.transpose(1, 0, 2)                       # V[d,u,e]+V[e,u,d]
    dd = np.arange(D)
    Vdiag = V[dd, :, dd]                                    # [d, u]

    ones = np.ones((P, B), dtype=BF16)
    zeros = np.zeros((P, B), dtype=BF16)

    def unit_block(M):      # [d, u] -> [p, k, u]
        return M.reshape(KC, P, U).transpose(1, 0, 2)

    in_maps = []
    for c in range(N_CORES):
        Sh_np = np.zeros((NJ, P, KC, U), dtype=np.float32)
        for j in range(NJ):
            o = 32 * c + j
            M = Vdiag if o == 0 else Ssum[dd, (dd + o) % D, :]
            Sh_np[j] = unit_block(M)

        mini_s = np.zeros((P, U), dtype=np.float32)
        if c < 4:                       # linear term, chunk c
            mA = xT_bf[P * c: P * (c + 1)]
            mB = ones
            mini_s = W[P * c: P * (c + 1), :].astype(np.float32)
        elif c == 4:                    # bias as rank-1 with ones
            mA = ones
            mB = ones
            mini_s[0, :] = b
        elif c == 5:                    # idle
            mA = zeros
            mB = zeros
        else:                           # o=256 pairs, halves on 6 and 7
            d0 = P * (c - 6)
            mA = xT_bf[d0: d0 + P]
            mB = xT_bf[d0 + 256: d0 + 256 + P]
            mini_s = Ssum[dd[d0:d0 + P], dd[d0:d0 + P] + 256, :]

        in_maps.append({
            "xTc": xTc_np,
            "xE": np.ascontiguousarray(xT_ext[32 * c: 32 * c + 544]),
            "Sh": Sh_np.astype(BF16),
            "minA": np.ascontiguousarray(mA),
            "minB": np.ascontiguousarray(mB),
            "shMini": mini_s.astype(BF16),
        })
    return in_maps


_LAST_RUN = {}


def kernel(x, W, V, b):
    _ensure_axon_hooks()
    import concourse.bass_utils as bass_utils
    bass_utils.upload_artifacts = lambda d: f"local:{d}"

    x = np.asarray(x, dtype=np.float32)
    W = np.asarray(W, dtype=np.float32)
    V = np.asarray(V, dtype=np.float32)
    b = np.asarray(b, dtype=np.float32)

    in_maps = _host_inputs(x, W, V, b)

    nc = _build_program()
    res = None
    last_exc = None
    for attempt in range(3):
        try:
            res = bass_utils.run_bass_kernel_spmd(
                nc, in_maps, core_ids=list(range(N_CORES)))
            break
        except Exception as e:  # transient NRT device errors have been seen
            last_exc = e
    if res is None:
        raise last_exc
    _LAST_RUN["result"] = res

    acc = np.zeros((U, B), dtype=np.float64)
    for c in range(N_CORES):
        acc += res.results[c]["outs"]
    return np.ascontiguousarray(acc.T).astype(np.float32)


# revision 7
# speedup vs baseline: 1.8994x; 1.0029x over previous
"""Trainium2 Bass kernel for nn_DenseTensor (dense_mlp, bilinear form).

Computes out = x @ W + einsum('bd,due,be->bu', x, V, x) + b with
B=1024, D=U=E=512 on 8 NeuronCores.

Algorithm: the quadratic form depends only on the symmetric part of V
in (d,e), so the D*E contraction folds to D(D+1)/2 unordered pairs
enumerated by circulant offset o: pair (d, e=(d+o)%512) for o in
[0,256].  Host pre-folds coefficients Sh[(o,d),u] = V[d,u,e]+V[e,u,d]
(o=0 diag: V[d,u,d]).  This HALVES the PE FLOPs vs the naive einsum.

Sharding: by contraction - core c owns offsets o in [32c, 32c+32);
every core computes a partial full [U,B] output and the host sums the
8 partials (the unshard step for contraction sharding).  Leftover work
rides one extra single-chunk "mini" unit per core, SPMD-uniform with
per-core data only:
  cores 0-3 : linear term chunk  (minA = x^T rows, minB = ones,
              coeff = W rows)
  core  4   : bias as rank-1     (minA = minB = ones, coeff row0 = b)
  core  5   : idle (zero coeffs)
  cores 6,7 : the o=256 half-offset pairs
Per-core, per o-unit:
  DMA : xw = 512-row window of the wrap-extended x^T at offset o
        (contiguous slabs - no on-device shuffles) + coeff block.
  DVE : G = xT .* xw  (pair products, one bf16 tensor_mul)
  PE  : 4 k-chunks x 4 u-blocks x 2 b-halves matmuls (N=512)
        accumulating into 8 PSUM banks across the whole kernel.
Tail: last unit loops accumulator-major so PSUM banks finish
staggered; evac copies alternate Vector/Scalar and stream out per
slice.  129 chunks/core = 1032 matmuls ~ 223 us of pure PE at the
warm roofline (216 ns per 128x128x512 bf16 matmul).
"""

import sys
import types

import numpy as np
import ml_dtypes

B, D, U = 1024, 512, 512
N_CORES = 8
P = 128                  # partitions
KC = D // P              # k-chunks per unit = 4
NJ = 32                  # o-units per core

BF16 = ml_dtypes.bfloat16


def _ensure_axon_hooks():
    """Provide the antenv.axon_hooks registry if the image lacks it."""
    try:
        import antenv.axon_hooks  # noqa: F401
        return
    except ImportError:
        pass
    mod = types.ModuleType("antenv.axon_hooks")
    mod._hook = None

    def set_axon_ntff_profile_hook(h):
        mod._hook = h

    def get_axon_ntff_profile_hook():
        return mod._hook

    mod.set_axon_ntff_profile_hook = set_axon_ntff_profile_hook
    mod.get_axon_ntff_profile_hook = get_axon_ntff_profile_hook
    sys.modules["antenv.axon_hooks"] = mod
    try:
        import antenv
        antenv.axon_hooks = mod
    except ImportError:
        pass
    try:
        from trn_agent_boot.trn_boot import _ntff_profile_via_ctypes
        hook = _ntff_profile_via_ctypes("/opt/axon/libaxon_pjrt.so")
        if hook is not None:
            set_axon_ntff_profile_hook(hook)
    except Exception:
        pass


def _split_multi_waits(nc, mybir, max_waits=1):
    """Legalize for walrus builds that allow only one sync wait per
    instruction: move extra waits onto same-engine NoOps placed just
    before the offending instruction (queues are in-order, so this is
    semantics-preserving)."""
    for f in nc.m.functions:
        for blk in f.blocks:
            new_insts, changed = [], False
            for inst in blk.instructions:
                si = inst.sync_info
                if si is not None and len(si.on_wait) > max_waits:
                    waits = list(si.on_wait)
                    extra, keep = waits[:-max_waits], waits[-max_waits:]
                    for j, w in enumerate(extra):
                        new_insts.append(mybir.InstNoOp(
                            name=f"{inst.name}-sw{j}",
                            engine=inst.engine,
                            bass_nofuse=True,
                            sync_info=mybir.SyncInfo(on_wait=[w], on_update=[]),
                        ))
                    inst.sync_info = mybir.SyncInfo(
                        on_wait=keep, on_update=list(si.on_update))
                    changed = True
                new_insts.append(inst)
            if changed:
                blk.instructions = new_insts


def _build_program():
    import concourse.bass as bass
    import concourse.mybir as mybir
    import concourse.tile as tile

    f32 = mybir.dt.float32
    bf16 = mybir.dt.bfloat16
    Copy = mybir.ActivationFunctionType.Copy

    nc = bass.Bass(trn_type="TRN2")
    xTc = nc.dram_tensor("xTc", [P, KC, B], bf16, kind="ExternalInput")
    xE = nc.dram_tensor("xE", [544, B], bf16, kind="ExternalInput")
    Sh = nc.dram_tensor("Sh", [NJ, P, KC, U], bf16, kind="ExternalInput")
    minA = nc.dram_tensor("minA", [P, B], bf16, kind="ExternalInput")
    minB = nc.dram_tensor("minB", [P, B], bf16, kind="ExternalInput")
    shMini = nc.dram_tensor("shMini", [P, U], bf16, kind="ExternalInput")
    outs = nc.dram_tensor("outs", [U, B], f32, kind="ExternalOutput")

    with tile.TileContext(nc) as tc:
        with tc.tile_pool(name="const", bufs=1) as cpool:
            xT_sb = cpool.tile([P, KC, B], bf16)
            out_sb = cpool.tile([P, KC, B], f32)
            ma_sb = cpool.tile([P, B], bf16)
            mb_sb = cpool.tile([P, B], bf16)
            ms_sb = cpool.tile([P, U], bf16)
            gm_sb = cpool.tile([P, B], bf16)

            # mini unit first: tiny loads so the PE starts almost
            # immediately while the big unit-0 windows stream in.
            nc.sync.dma_start(out=ma_sb, in_=minA[:, :])
            nc.sync.dma_start(out=mb_sb, in_=minB[:, :])
            nc.scalar.dma_start(out=ms_sb, in_=shMini[:, :])

            with tc.tile_pool(name="wp", bufs=5) as wpool, \
                 tc.tile_pool(name="gp", bufs=4) as gpool, \
                 tc.tile_pool(name="sp", bufs=6) as spool, \
                 tc.tile_pool(name="ap", bufs=1, space="PSUM") as apool:
                accs = [[None, None] for _ in range(4)]
                for ub in range(4):
                    for h in range(2):
                        acc_t = apool.tile([P, 512], f32, tag=f"acc{ub}_{h}")
                        accs[ub][h] = acc_t

                nc.vector.tensor_mul(gm_sb, ma_sb, mb_sb)
                for ub in range(4):
                    for h in range(2):
                        nc.tensor.matmul(
                            accs[ub][h],
                            ms_sb[:, ub * P:(ub + 1) * P],
                            gm_sb[:, h * 512:(h + 1) * 512],
                            start=True, stop=False)

                # per-k DMA/TT granularity everywhere: window chunks
                # k=0,1 ride the sync (HWDGE) queue, k=2,3 the gpsimd
                # (SWDGE) queue, coefficients the scalar queue - three
                # DMA paths feed the PE in parallel and each matmul
                # only waits for its own k-chunk.
                for k in range(KC):
                    nc.gpsimd.dma_start(out=xT_sb[:, k, :], in_=xTc[:, k, :])
                for j in range(NJ):
                    xw = wpool.tile([P, KC, B], bf16, tag="xw")
                    g = gpool.tile([P, KC, B], bf16, tag="g")
                    sh = spool.tile([P, KC, U], bf16, tag="sh")
                    for k in range(KC):
                        eng = nc.sync if k < 2 else nc.gpsimd
                        eng.dma_start(
                            out=xw[:, k, :],
                            in_=xE[j + P * k: j + P * (k + 1), :])
                        nc.scalar.dma_start(
                            out=sh[:, k, :], in_=Sh[j, :, k, :])
                        nc.vector.tensor_mul(
                            g[:, k, :], xT_sb[:, k, :], xw[:, k, :])
                    if j < NJ - 1:
                        for k in range(KC):
                            for ub in range(4):
                                for h in range(2):
                                    nc.tensor.matmul(
                                        accs[ub][h],
                                        sh[:, k, ub * P:(ub + 1) * P],
                                        g[:, k, h * 512:(h + 1) * 512],
                                        start=False, stop=False)
                    else:
                        # last unit: accumulator-major so PSUM banks
                        # retire staggered and evac overlaps the tail.
                        for ub in range(4):
                            for h in range(2):
                                for k in range(KC):
                                    nc.tensor.matmul(
                                        accs[ub][h],
                                        sh[:, k, ub * P:(ub + 1) * P],
                                        g[:, k, h * 512:(h + 1) * 512],
                                        start=False, stop=(k == KC - 1))

                outs_r = outs.rearrange("(ub p) b -> p ub b", p=P)
                i = 0
                for ub in range(4):
                    for h in range(2):
                        dst = out_sb[:, ub, h * 512:(h + 1) * 512]
                        if i % 2 == 0:
                            nc.vector.tensor_copy(dst, accs[ub][h])
                        else:
                            nc.scalar.activation(dst, accs[ub][h], Copy)
                        nc.sync.dma_start(
                            out=outs_r[:, ub, h * 512:(h + 1) * 512],
                            in_=dst)
                        i += 1

    _split_multi_waits(nc, mybir, max_waits=1)
    return nc


def _host_inputs(x, W, V, b):
    """Build the per-core input arrays (all host-side prep)."""
    xT_bf = np.ascontiguousarray(x.T).astype(BF16)          # [D, B]
    xT_ext = np.concatenate([xT_bf, xT_bf[:256]], axis=0)   # [768, B]
    xTc_np = np.ascontiguousarray(
        xT_bf.reshape(KC, P, B).transpose(1, 0, 2))         # [P, KC, B]

    # folded symmetric coefficients
    Vt = V.transpose(0, 2, 1)                               # [d, e, u]
    Ssum = Vt + Vt.transpose(1, 0, 2)                       # V[d,u,e]+V[e,u,d]
    dd = np.arange(D)
    Vdiag = V[dd, :, dd]                                    # [d, u]

    ones = np.ones((P, B), dtype=BF16)
    zeros = np.zeros((P, B), dtype=BF16)

    def unit_block(M):      # [d, u] -> [p, k, u]
        return M.reshape(KC, P, U).transpose(1, 0, 2)

    in_maps = []
    for c in range(N_CORES):
        Sh_np = np.zeros((NJ, P, KC, U), dtype=np.float32)
        for j in range(NJ):
            o = 32 * c + j
            M = Vdiag if o == 0 else Ssum[dd, (dd + o) % D, :]
            Sh_np[j] = unit_block(M)

        mini_s = np.zeros((P, U), dtype=np.float32)
        if c < 4:                       # linear term, chunk c
            mA = xT_bf[P * c: P * (c + 1)]
            mB = ones
            mini_s = W[P * c: P * (c + 1), :].astype(np.float32)
        elif c == 4:                    # bias as rank-1 with ones
            mA = ones
            mB = ones
            mini_s[0, :] = b
        elif c == 5:                    # idle
            mA = zeros
            mB = zeros
        else:                           # o=256 pairs, halves on 6 and 7
            d0 = P * (c - 6)
            mA = xT_bf[d0: d0 + P]
            mB = xT_bf[d0 + 256: d0 + 256 + P]
            mini_s = Ssum[dd[d0:d0 + P], dd[d0:d0 + P] + 256, :]

        in_maps.append({
            "xTc": xTc_np,
            "xE": np.ascontiguousarray(xT_ext[32 * c: 32 * c + 544]),
            "Sh": Sh_np.astype(BF16),
            "minA": np.ascontiguousarray(mA),
            "minB": np.ascontiguousarray(mB),
            "shMini": mini_s.astype(BF16),
        })
    return in_maps


_LAST_RUN = {}


def kernel(x, W, V, b):
    _ensure_axon_hooks()
    import concourse.bass_utils as bass_utils
    bass_utils.upload_artifacts = lambda d: f"local:{d}"

    x = np.asarray(x, dtype=np.float32)
    W = np.asarray(W, dtype=np.float32)
    V = np.asarray(V, dtype=np.float32)
    b = np.asarray(b, dtype=np.float32)

    in_maps = _host_inputs(x, W, V, b)

    nc = _build_program()
    res = None
    last_exc = None
    for attempt in range(3):
        try:
            res = bass_utils.run_bass_kernel_spmd(
                nc, in_maps, core_ids=list(range(N_CORES)))
            break
        except Exception as e:  # transient NRT device errors have been seen
            last_exc = e
    if res is None:
        raise last_exc
    _LAST_RUN["result"] = res

    acc = np.zeros((U, B), dtype=np.float64)
    for c in range(N_CORES):
        acc += res.results[c]["outs"]
    return np.ascontiguousarray(acc.T).astype(np.float32)


# revision 8
# speedup vs baseline: 1.9413x; 1.0221x over previous
"""Trainium2 Bass kernel for nn_DenseTensor (dense_mlp, bilinear form).

Computes out = x @ W + einsum('bd,due,be->bu', x, V, x) + b with
B=1024, D=U=E=512 on 8 NeuronCores.

Algorithm: the quadratic form depends only on the symmetric part of V
in (d,e), so the D*E contraction folds to D(D+1)/2 unordered pairs
enumerated by circulant offset o: pair (d, e=(d+o)%512) for o in
[0,256].  Host pre-folds coefficients Sh[(o,d),u] = V[d,u,e]+V[e,u,d]
(o=0 diag: V[d,u,d]).  This HALVES the PE FLOPs vs the naive einsum.

Sharding: by contraction - core c owns offsets o in [32c, 32c+32);
every core computes a partial full [U,B] output and the host sums the
8 partials (the unshard step for contraction sharding).  Leftover work
rides one extra single-chunk "mini" unit per core, SPMD-uniform with
per-core data only:
  cores 0-3 : linear term chunk  (minA = x^T rows, minB = ones,
              coeff = W rows)
  core  4   : bias as rank-1     (minA = minB = ones, coeff row0 = b)
  core  5   : idle (zero coeffs)
  cores 6,7 : the o=256 half-offset pairs
Per-core, per o-unit:
  DMA : xw = 512-row window of the wrap-extended x^T at offset o
        (contiguous slabs - no on-device shuffles) + coeff block.
  DVE : G = xT .* xw  (pair products, one bf16 tensor_mul)
  PE  : 4 k-chunks x 4 u-blocks x 2 b-halves matmuls (N=512)
        accumulating into 8 PSUM banks across the whole kernel.
Tail: last unit loops accumulator-major so PSUM banks finish
staggered; evac copies alternate Vector/Scalar and stream out per
slice.  129 chunks/core = 1032 matmuls ~ 223 us of pure PE at the
warm roofline (216 ns per 128x128x512 bf16 matmul).
"""

import sys
import types

import numpy as np
import ml_dtypes

B, D, U = 1024, 512, 512
N_CORES = 8
P = 128                  # partitions
KC = D // P              # k-chunks per unit = 4
NJ = 32                  # o-units per core

BF16 = ml_dtypes.bfloat16


def _ensure_axon_hooks():
    """Provide the antenv.axon_hooks registry if the image lacks it."""
    try:
        import antenv.axon_hooks  # noqa: F401
        return
    except ImportError:
        pass
    mod = types.ModuleType("antenv.axon_hooks")
    mod._hook = None

    def set_axon_ntff_profile_hook(h):
        mod._hook = h

    def get_axon_ntff_profile_hook():
        return mod._hook

    mod.set_axon_ntff_profile_hook = set_axon_ntff_profile_hook
    mod.get_axon_ntff_profile_hook = get_axon_ntff_profile_hook
    sys.modules["antenv.axon_hooks"] = mod
    try:
        import antenv
        antenv.axon_hooks = mod
    except ImportError:
        pass
    try:
        from trn_agent_boot.trn_boot import _ntff_profile_via_ctypes
        hook = _ntff_profile_via_ctypes("/opt/axon/libaxon_pjrt.so")
        if hook is not None:
            set_axon_ntff_profile_hook(hook)
    except Exception:
        pass


def _split_multi_waits(nc, mybir, max_waits=1):
    """Legalize for walrus builds that allow only one sync wait per
    instruction: move extra waits onto same-engine NoOps placed just
    before the offending instruction (queues are in-order, so this is
    semantics-preserving)."""
    for f in nc.m.functions:
        for blk in f.blocks:
            new_insts, changed = [], False
            for inst in blk.instructions:
                si = inst.sync_info
                if si is not None and len(si.on_wait) > max_waits:
                    waits = list(si.on_wait)
                    extra, keep = waits[:-max_waits], waits[-max_waits:]
                    for j, w in enumerate(extra):
                        new_insts.append(mybir.InstNoOp(
                            name=f"{inst.name}-sw{j}",
                            engine=inst.engine,
                            bass_nofuse=True,
                            sync_info=mybir.SyncInfo(on_wait=[w], on_update=[]),
                        ))
                    inst.sync_info = mybir.SyncInfo(
                        on_wait=keep, on_update=list(si.on_update))
                    changed = True
                new_insts.append(inst)
            if changed:
                blk.instructions = new_insts


def _build_program():
    import concourse.bass as bass
    import concourse.mybir as mybir
    import concourse.tile as tile

    f32 = mybir.dt.float32
    bf16 = mybir.dt.bfloat16
    Copy = mybir.ActivationFunctionType.Copy

    nc = bass.Bass(trn_type="TRN2")
    xTc = nc.dram_tensor("xTc", [P, KC, B], bf16, kind="ExternalInput")
    xE = nc.dram_tensor("xE", [544, B], bf16, kind="ExternalInput")
    Sh = nc.dram_tensor("Sh", [NJ, P, KC, U], bf16, kind="ExternalInput")
    minA = nc.dram_tensor("minA", [P, B], bf16, kind="ExternalInput")
    minB = nc.dram_tensor("minB", [P, B], bf16, kind="ExternalInput")
    shMini = nc.dram_tensor("shMini", [P, U], bf16, kind="ExternalInput")
    outs = nc.dram_tensor("outs", [U, B], f32, kind="ExternalOutput")

    with tile.TileContext(nc) as tc:
        with tc.tile_pool(name="const", bufs=1) as cpool:
            xT_sb = cpool.tile([P, KC, B], bf16)
            out_sb = cpool.tile([P, KC, B], f32)
            ma_sb = cpool.tile([P, B], bf16)
            mb_sb = cpool.tile([P, B], bf16)
            ms_sb = cpool.tile([P, U], bf16)
            gm_sb = cpool.tile([P, B], bf16)

            # mini unit first: tiny loads so the PE starts almost
            # immediately while the big unit-0 windows stream in.
            nc.sync.dma_start(out=ma_sb, in_=minA[:, :])
            nc.sync.dma_start(out=mb_sb, in_=minB[:, :])
            nc.scalar.dma_start(out=ms_sb, in_=shMini[:, :])

            with tc.tile_pool(name="wp", bufs=5) as wpool, \
                 tc.tile_pool(name="gp", bufs=4) as gpool, \
                 tc.tile_pool(name="sp", bufs=6) as spool, \
                 tc.tile_pool(name="ap", bufs=1, space="PSUM") as apool:
                accs = [[None, None] for _ in range(4)]
                for ub in range(4):
                    for h in range(2):
                        acc_t = apool.tile([P, 512], f32, tag=f"acc{ub}_{h}")
                        accs[ub][h] = acc_t

                nc.vector.tensor_mul(gm_sb, ma_sb, mb_sb)
                for ub in range(4):
                    for h in range(2):
                        nc.tensor.matmul(
                            accs[ub][h],
                            ms_sb[:, ub * P:(ub + 1) * P],
                            gm_sb[:, h * 512:(h + 1) * 512],
                            start=True, stop=False)

                # per-k DMA/TT granularity everywhere, spread over the
                # two HWDGE rings (sync ~25MB, scalar ~25MB) so each
                # matmul only waits for its own k-chunk and neither
                # queue becomes the critical path.
                for j in range(NJ):
                    xw = wpool.tile([P, KC, B], bf16, tag="xw")
                    g = gpool.tile([P, KC, B], bf16, tag="g")
                    sh = spool.tile([P, KC, U], bf16, tag="sh")
                    for k in range(KC):
                        eng = nc.sync if k < 3 else nc.scalar
                        eng.dma_start(
                            out=xw[:, k, :],
                            in_=xE[j + P * k: j + P * (k + 1), :])
                        if j == 0:
                            nc.scalar.dma_start(
                                out=xT_sb[:, k, :], in_=xTc[:, k, :])
                        nc.scalar.dma_start(
                            out=sh[:, k, :], in_=Sh[j, :, k, :])
                        nc.vector.tensor_mul(
                            g[:, k, :], xT_sb[:, k, :], xw[:, k, :])
                    if j < NJ - 1:
                        for k in range(KC):
                            for ub in range(4):
                                for h in range(2):
                                    nc.tensor.matmul(
                                        accs[ub][h],
                                        sh[:, k, ub * P:(ub + 1) * P],
                                        g[:, k, h * 512:(h + 1) * 512],
                                        start=False, stop=False)
                    else:
                        # last unit: accumulator-major so PSUM banks
                        # retire staggered and evac overlaps the tail.
                        for ub in range(4):
                            for h in range(2):
                                for k in range(KC):
                                    nc.tensor.matmul(
                                        accs[ub][h],
                                        sh[:, k, ub * P:(ub + 1) * P],
                                        g[:, k, h * 512:(h + 1) * 512],
                                        start=False, stop=(k == KC - 1))

                outs_r = outs.rearrange("(ub p) b -> p ub b", p=P)
                i = 0
                for ub in range(4):
                    for h in range(2):
                        dst = out_sb[:, ub, h * 512:(h + 1) * 512]
                        if i % 2 == 0:
                            nc.vector.tensor_copy(dst, accs[ub][h])
                        else:
                            nc.scalar.activation(dst, accs[ub][h], Copy)
                        nc.sync.dma_start(
                            out=outs_r[:, ub, h * 512:(h + 1) * 512],
                            in_=dst)
                        i += 1

    _split_multi_waits(nc, mybir, max_waits=1)
    return nc


def _host_inputs(x, W, V, b):
    """Build the per-core input arrays (all host-side prep)."""
    xT_bf = np.ascontiguousarray(x.T).astype(BF16)          # [D, B]
    xT_ext = np.concatenate([xT_bf, xT_bf[:256]], axis=0)   # [768, B]
    xTc_np = np.ascontiguousarray(
        xT_bf.reshape(KC, P, B).transpose(1, 0, 2))         # [P, KC, B]

    # folded symmetric coefficients
    Vt = V.transpose(0, 2, 1)                               # [d, e, u]
    Ssum = Vt + Vt.transpose(1, 0, 2)                       # V[d,u,e]+V[e,u,d]
    dd = np.arange(D)
    Vdiag = V[dd, :, dd]                                    # [d, u]

    ones = np.ones((P, B), dtype=BF16)
    zeros = np.zeros((P, B), dtype=BF16)

    def unit_block(M):      # [d, u] -> [p, k, u]
        return M.reshape(KC, P, U).transpose(1, 0, 2)

    in_maps = []
    for c in range(N_CORES):
        Sh_np = np.zeros((NJ, P, KC, U), dtype=np.float32)
        for j in range(NJ):
            o = 32 * c + j
            M = Vdiag if o == 0 else Ssum[dd, (dd + o) % D, :]
            Sh_np[j] = unit_block(M)

        mini_s = np.zeros((P, U), dtype=np.float32)
        if c < 4:                       # linear term, chunk c
            mA = xT_bf[P * c: P * (c + 1)]
            mB = ones
            mini_s = W[P * c: P * (c + 1), :].astype(np.float32)
        elif c == 4:                    # bias as rank-1 with ones
            mA = ones
            mB = ones
            mini_s[0, :] = b
        elif c == 5:                    # idle
            mA = zeros
            mB = zeros
        else:                           # o=256 pairs, halves on 6 and 7
            d0 = P * (c - 6)
            mA = xT_bf[d0: d0 + P]
            mB = xT_bf[d0 + 256: d0 + 256 + P]
            mini_s = Ssum[dd[d0:d0 + P], dd[d0:d0 + P] + 256, :]

        in_maps.append({
            "xTc": xTc_np,
            "xE": np.ascontiguousarray(xT_ext[32 * c: 32 * c + 544]),
            "Sh": Sh_np.astype(BF16),
            "minA": np.ascontiguousarray(mA),
            "minB": np.ascontiguousarray(mB),
            "shMini": mini_s.astype(BF16),
        })
    return in_maps


_LAST_RUN = {}


def kernel(x, W, V, b):
    _ensure_axon_hooks()
    import concourse.bass_utils as bass_utils
    bass_utils.upload_artifacts = lambda d: f"local:{d}"

    x = np.asarray(x, dtype=np.float32)
    W = np.asarray(W, dtype=np.float32)
    V = np.asarray(V, dtype=np.float32)
    b = np.asarray(b, dtype=np.float32)

    in_maps = _host_inputs(x, W, V, b)

    nc = _build_program()
    res = None
    last_exc = None
    for attempt in range(3):
        try:
            res = bass_utils.run_bass_kernel_spmd(
                nc, in_maps, core_ids=list(range(N_CORES)))
            break
        except Exception as e:  # transient NRT device errors have been seen
            last_exc = e
    if res is None:
        raise last_exc
    _LAST_RUN["result"] = res

    acc = np.zeros((U, B), dtype=np.float64)
    for c in range(N_CORES):
        acc += res.results[c]["outs"]
    return np.ascontiguousarray(acc.T).astype(np.float32)
